# revision 94
# baseline (speedup 1.0000x reference)
"""Trainium2 Bass kernel for nn_CNNModel_76312978915482.

Computation (bit-identical to the CPU-jax f32 reference on the dataset):
  conv  = 2x2 all-ones conv, stride 2, pad 1 on x [B,1,330,314] -> [B,1,166,158]
          summed as (x00+x01)+(x10+x11)  (XLA CPU order)
  m     = min(conv, 0) min-pooled 2x2      ( == -maxpool(|min(conv,0)|), exact)
  s     = conv sum-pooled 2x2 as (c00+c10)+(c01+c11) (vertical-first pairing;
          0 condition flips vs the reference's serial order on the dataset,
          margins are ~45 ulps)
  cond  = (m < lb) & ((s/4)/m > q1/lb)  evaluated as the product compare
          NOT cond = (m >= lb) | (s >= fl(4thr*m))
  out[r,c] = 1 - cond[(r+1)//4 clip, (c+1)//4 clip]   (structured scatter)

Layout: pure data parallel, batch 256 -> 32 images per core x 8 cores.
The host zero-pads each image to [332, 316]; a padded image is exactly 83
contiguous blocks of 4*316 floats (block I = pooled row I).  Per core the
32*83 = 2656 job blocks are streamed through 10 tiles (taper 1,2,4,4,4,2,
1,1,1 x128 + 1x96 jobs per partition).  The host permutes the job stream
so that stream slot (p, q) of EVERY tile processes the same pooled row
(an exact-cover assignment of rows to slots); the per-cell lb / 4*thr
tables are therefore tile-invariant [128, 4*79] instead of per-tile,
cutting their DMA cost 8x.  The output mask is computed at full
resolution on-device (the 4x4 scatter expansion runs on-chip) and stored
as uint8 0/1; the host gather de-permutes, strips padding, and casts the
full-shape array to float32.

In the cost model every DMA transfer serializes on one DMA-engine device
at ~360 B/ns, so the floor is set by bytes moved: 13.3 MB f32 loads (pad
columns are zeroed on-chip, not shipped) + 3.4 MB u8 stores + 0.3 MB
tables = ~47.3 us.  Compute is spread so no
engine exceeds the DMA rhythm, using only ISA-legal op/engine pairs
(Pool TensorTensor supports only add/mult; min/is_ge/max and mixed-dtype
writes are DVE-only; Act does single-input copies with dtype convert):
  DVE  (~28 us): hp rows 0-1, conv row 0, min tree, product compare
  Pool (~34 us): hp rows 2-3, conv row 1, sum tree (adds/mult only)
  Act  (~18 us): ov -> u8 x4-column-replicate copy, u16 row-replicate,
                 store issue; SP issues loads
The last tile's output chain runs fused on DVE (legal mixed-dtype max)
to skip the Act queue at the drain.  Output blocks accumulate in one
static SBUF buffer (no buffer-ring reuse waits) and stores are merged
across tiles so all 18 DMAs fit the 8 hardware queues with trivially
satisfied reuse chains.  Measured makespan: 52885 ns (baseline 86066).
"""
import numpy as np

B, H, W = 256, 330, 314
Hp, Wp = 83, 79
NCORES = 8
BC = B // NCORES          # images per core (32)
H2, W2 = H + 2, W + 2     # padded image (332, 316)
BLK = 4 * W2              # f32 elements per SBUF job block (1264)
BLKF = 4 * W            # f32 elements per stream job block (pad cols dropped)
BLK8 = 4 * W2             # u8 bytes per output job block (1264)
HJ = W2 // 2              # conv cols (158)
NJOB = BC * Hp            # jobs per core (2656)
JPP = 4                   # max jobs per partition per tile
TILES = [(1, 128), (2, 128), (4, 128), (4, 128), (4, 128), (2, 128),
         (1, 128), (1, 128), (1, 128), (1, 96)]
assert sum(q * p for q, p in TILES) == NJOB

_CACHE: dict = {}


def _build_layout():
    """Solve the slot->row exact cover and the stream permutation.

    act[p, q] = number of tiles in which stream slot (p, q) is active.
    Each pooled row r must receive exactly BC jobs:  sum of act over its
    slots == 32.  The decomposition below consumes the act-value supply
    {10 x96, 9 x32, 5 x128, 3 x256} exactly.
    Returns (slot_row[128, JPP], stream_b[NJOB], stream_r[NJOB]).
    """
    act = np.zeros((128, JPP), np.int64)
    for q_n, P in TILES:
        act[:P, :q_n] += 1
    # patterns: list of (row_count, [act values consumed per row])
    patterns = [
        (32, [10, 10, 9, 3]),
        (32, [10, 5, 5, 3, 3, 3, 3]),
        (15, [5, 5, 5, 5, 3, 3, 3, 3]),
        (4, [5, 3, 3, 3, 3, 3, 3, 3, 3, 3]),
    ]
    assert sum(n for n, _ in patterns) == Hp
    assert all(sum(pat) == BC for _, pat in patterns)
    slots_by_val: dict[int, list] = {}
    for p in range(128):
        for q in range(JPP):
            slots_by_val.setdefault(int(act[p, q]), []).append((p, q))
    slot_row = -np.ones((128, JPP), np.int64)
    r = 0
    for n, pat in patterns:
        for _ in range(n):
            for v in pat:
                p, q = slots_by_val[v].pop()
                slot_row[p, q] = r
            r += 1
    assert all(not v for v in slots_by_val.values()), "supply not exhausted"
    # per-row job totals check
    chk = np.zeros(Hp, np.int64)
    for p in range(128):
        for q in range(JPP):
            chk[slot_row[p, q]] += act[p, q]
    assert (chk == BC).all()

    # stream order within a tile is q-major then p (matches the load AP)
    img_counter = np.zeros(Hp, np.int64)
    stream_b = np.empty(NJOB, np.int64)
    stream_r = np.empty(NJOB, np.int64)
    base = 0
    for q_n, P in TILES:
        for q in range(q_n):
            for p in range(P):
                rr = slot_row[p, q]
                stream_b[base + q * P + p] = img_counter[rr]
                stream_r[base + q * P + p] = rr
                img_counter[rr] += 1
        base += q_n * P
    assert (img_counter == BC).all()
    # xp-stream job order: p-major within a tile so the load's source AP
    # fuses (q, r, c) into one contiguous run per partition
    xp_order = np.empty(NJOB, np.int64)
    base = 0
    i = 0
    for q_n, P in TILES:
        for p in range(P):
            for q in range(q_n):
                xp_order[i] = base + q * P + p
                i += 1
        base += q_n * P
    return slot_row, stream_b, stream_r, xp_order


def get_layout():
    if "layout" not in _CACHE:
        _CACHE["layout"] = _build_layout()
    return _CACHE["layout"]


def make_tables(lb, thr4):
    """lb/4*thr [Hp, Wp] -> tile-invariant [128, JPP*Wp] slot tables."""
    slot_row = get_layout()[0]
    lbt = np.empty((128, JPP * Wp), np.float32)
    tht = np.empty((128, JPP * Wp), np.float32)
    for q in range(JPP):
        lbt[:, q * Wp:(q + 1) * Wp] = lb[slot_row[:, q]]
        tht[:, q * Wp:(q + 1) * Wp] = thr4[slot_row[:, q]]
    return lbt, tht


def make_stream(x):
    """[n,1,H,W] (or [n,H,W]) f32 -> permuted flat job stream [NJOB*BLK]."""
    if x.ndim == 4:
        x = x[:, 0]
    _, stream_b, stream_r, xp_order = get_layout()
    xp = np.zeros((x.shape[0], H2, W2), np.float32)
    xp[:, 1:H + 1, 1:W + 1] = x
    xpb = np.ascontiguousarray(
        xp.reshape(x.shape[0], Hp, 4, W2)[:, :, :, 1:W + 1]
    ).reshape(x.shape[0], Hp, BLKF)
    return np.ascontiguousarray(
        xpb[stream_b[xp_order], stream_r[xp_order]].reshape(-1))


def unstream_out(out_flat):
    """flat u8 job stream [NJOB*BLK8] -> [BC, H, W] float32."""
    _, stream_b, stream_r, _ = get_layout()
    outb = np.empty((BC, Hp, BLK8), np.uint8)
    outb[stream_b, stream_r] = out_flat.reshape(NJOB, BLK8)
    full = outb.reshape(BC, H2, W2)[:, 1:H + 1, 1:W + 1]
    return full.astype(np.float32)


def _build_nc():
    import concourse.bacc as bacc
    import concourse.mybir as mybir
    import concourse.tile as tile

    dt = mybir.dt.float32
    u8 = mybir.dt.uint8
    u16 = mybir.dt.uint16
    u32 = mybir.dt.uint32
    A = mybir.AluOpType

    nc = bacc.Bacc("TRN2", target_bir_lowering=False, debug=False)
    xp_d = nc.dram_tensor("xp", [NJOB * BLKF], dt, kind="ExternalInput")
    lbx_d = nc.dram_tensor("lbx", [128, JPP * Wp], dt, kind="ExternalInput")
    thrx_d = nc.dram_tensor("thrx", [128, JPP * Wp], dt, kind="ExternalInput")
    out_d = nc.dram_tensor("out", [NJOB * BLK8], u8, kind="ExternalOutput")

    NSLOT = sum(q for q, _ in TILES)   # total job slots per partition (21)

    with tile.TileContext(nc) as tc:
        with tc.tile_pool(name="const", bufs=1) as cpool, \
             tc.tile_pool(name="bigx", bufs=6) as xpool, \
             tc.tile_pool(name="big", bufs=2) as bpool, \
             tc.tile_pool(name="small", bufs=2) as spool:
        # constants ride the (initially idle) Activation HWDGE ring
            lbt = cpool.tile([128, JPP * Wp], dt)
            thrt = cpool.tile([128, JPP * Wp], dt)
            # the table loads ride the Pool SWDGE path (DMASW lanes): that
            # keeps the HWDGE count at exactly 16, so the 8 hardware queues
            # pair loads only with loads and no load ever waits on a store
            nc.gpsimd.dma_start(lbt[:, :], lbx_d[:, :])
            nc.gpsimd.dma_start(thrt[:, :], thrx_d[:, :])
            # one static output buffer holds every tile's u8 blocks: no
            # buffer-ring reuse waits, and stores can merge across tiles
            obfull = cpool.tile([128, NSLOT * BLK8], u8)


            def build_tile(j0, P, jpp, s0, ti_idx=-1):
                dve_rowrep = ti_idx in DVE_ROWREP
                """Stage closures for one tile (P partitions x jpp jobs,
                stream jobs j0..).  Emitted in software-pipelined order so
                each engine's in-order instruction stream never waits on a
                same-round cross-engine producer."""
                st: dict = {}

                def load():
                    nel = P * jpp * BLKF
                    xt = xpool.tile([128, JPP * BLK], dt, tag="xt")
                    xv = xt[:, :].rearrange(
                        "p (q r c) -> p q r c", q=JPP, r=4, c=W2)
                    st["xv"] = xv
                    # loads carry only the 314 real columns; the two pad
                    # columns are zeroed by a tiny memset that runs off the
                    # critical path (it only waits on the buffer, not the
                    # load); descriptors stay >= 512B
                    nc.vector.memset(
                        xv[:P, :jpp, :, 0:W2:W2 - 1], 0.0)
                    nc.sync.dma_start(
                        xv[:P, :jpp, :, 1:W + 1],
                        xp_d[j0 * BLKF: j0 * BLKF + nel].rearrange(
                            "(p q r c) -> p q r c", p=P, q=jpp, r=4, c=W))

                def front():
                    # hp[q, r, j] = x[q, r, 2j] + x[q, r, 2j+1]    (DVE)
                    xv = st.pop("xv")
                    hp = bpool.tile([128, JPP * 4 * HJ], dt, tag="hp")
                    hpv = hp[:, :].rearrange(
                        "p (q r j) -> p q r j", q=JPP, r=4, j=HJ)
                    nc.vector.tensor_tensor(
                        hpv[:P, :jpp, 0:2], xv[:P, :jpp, 0:2, 0:W2:2],
                        xv[:P, :jpp, 0:2, 1:W2:2], A.add)
                    nc.gpsimd.tensor_tensor(
                        hpv[:P, :jpp, 2:4], xv[:P, :jpp, 2:4, 0:W2:2],
                        xv[:P, :jpp, 2:4, 1:W2:2], A.add)
                    st["hpv"] = hpv

                def small(tag, n=Wp):
                    tl = spool.tile([128, JPP * n], dt, tag=tag)
                    return tl[:, :].rearrange("p (q j) -> p q j", q=JPP)[:P, :jpp]

                def mid():
                    # sum tree on Pool: conv rows cv = hp_even + hp_odd,
                    # vt = cv0 + cv1, sv = vt_e + vt_o  (adds are Pool-legal)
                    hpv = st.pop("hpv")
                    cv = bpool.tile([128, JPP * 2 * HJ], dt, tag="cv")
                    cvv = cv[:, :].rearrange(
                        "p (q i j) -> p q i j", q=JPP, i=2, j=HJ)
                    nc.vector.tensor_tensor(
                        cvv[:P, :jpp, 0], hpv[:P, :jpp, 0, :],
                        hpv[:P, :jpp, 1, :], A.add)
                    nc.gpsimd.tensor_tensor(
                        cvv[:P, :jpp, 1], hpv[:P, :jpp, 2, :],
                        hpv[:P, :jpp, 3, :], A.add)
                    vt = small("vt", HJ)
                    nc.gpsimd.tensor_tensor(
                        vt, cvv[:P, :jpp, 0, :], cvv[:P, :jpp, 1, :], A.add)
                    sv = small("sv")
                    nc.gpsimd.tensor_tensor(
                        sv, vt[:, :, 0:HJ:2], vt[:, :, 1:HJ:2], A.add)
                    st["cvv"] = cvv
                    st["sv"] = sv

                def back():
                    # min tree and the product compare on DVE (min / is_ge
                    # are not Pool-legal); tm stays DVE to avoid a Pool
                    # round-trip for the mv operand
                    cvv = st.pop("cvv")
                    sv = st.pop("sv")
                    mh = small("mh", HJ)
                    nc.vector.tensor_tensor(
                        mh, cvv[:P, :jpp, 0, :], cvv[:P, :jpp, 1, :], A.min)
                    mv = small("mv")
                    nc.vector.scalar_tensor_tensor(
                        mv, mh[:, :, 0:HJ:2], 0.0, mh[:, :, 1:HJ:2],
                        A.min, A.min)
                    # o = 1 - (m<lb)&((s/4)/m>thr) = max(m>=lb, s>=fl(4thr*m))
                    # (product compare; thrt holds 4*thr)
                    sl = slice(0, jpp * Wp)
                    lbv = lbt[:P, sl].rearrange("p (q j) -> p q j", q=jpp)
                    thrv = thrt[:P, sl].rearrange("p (q j) -> p q j", q=jpp)
                    tm = small("tm")
                    nc.vector.tensor_tensor(tm, mv, thrv, A.mult)
                    nc1 = small("nc1")
                    nc.vector.tensor_tensor(nc1, mv, lbv, A.is_ge)
                    nc2 = small("nc2")
                    nc.vector.tensor_tensor(nc2, sv, tm, A.is_ge)
                    st["nc1"] = nc1
                    st["nc2"] = nc2

                def out():
                    # row0 u8: for normal tiles ov = max(nc1,nc2) on DVE then
                    # a f32->u8 x4-column-replicating copy on Act; the last
                    # tile fuses everything on DVE (mixed-dtype is legal
                    # there) to skip the Act queue at the drain
                    nc1 = st.pop("nc1")
                    nc2 = st.pop("nc2")
                    obv = obfull[:, s0 * BLK8:(s0 + jpp) * BLK8].rearrange(
                        "p (q r c) -> p q r c", q=jpp, r=4, c=W2)
                    row0 = obv[:P, :, 0, :].rearrange(
                        "p q (j k) -> p q j k", j=Wp, k=4)
                    if dve_rowrep:
                        nc.vector.tensor_tensor(
                            row0,
                            nc1.unsqueeze(3).broadcast_to([P, jpp, Wp, 4]),
                            nc2.unsqueeze(3).broadcast_to([P, jpp, Wp, 4]),
                            A.max)
                    else:
                        ov = small("ov")
                        nc.vector.tensor_tensor(ov, nc1, nc2, A.max)
                        nc.scalar.copy(
                            row0,
                            ov.unsqueeze(3).broadcast_to([P, jpp, Wp, 4]))
                    r0u = obv[:P, :, 0, :].bitcast(u16)
                    rest = obv[:P, :, 1:4, :].bitcast(u16)
                    if dve_rowrep:
                        nc.vector.tensor_copy(
                            rest,
                            r0u.unsqueeze(2).broadcast_to([P, jpp, 3, W2 // 2]))
                    else:
                        nc.scalar.copy(
                            rest,
                            r0u.unsqueeze(2).broadcast_to([P, jpp, 3, W2 // 2]))

                return load, front, mid, back, out

            # per-tile engine split: mid tiles pair DVE sum-tree with Pool
            # min-tree; tail tiles alternate whole chains across engines so
            # they drain concurrently
            DVE_ROWREP = {9}                   # last tile: no Act hop
            tiles = []
            tinfo = []
            j0 = 0
            s0 = 0
            for ti, (q_n, P) in enumerate(TILES):
                tiles.append(build_tile(j0, P, q_n, s0, ti))
                tinfo.append((j0, P, q_n, s0))
                j0 += q_n * P
                s0 += q_n

            # stores merged across consecutive same-P tiles: fewer DMAs
            # keeps every hardware DMA queue's reuse chain trivially
            # satisfied (no load ends up waiting on a store's completion)
            STORE_GROUPS = [[0, 1], [2, 3], [4, 5], [6, 7], [8], [9]]
            assert sorted(g for grp in STORE_GROUPS for g in grp) == \
                list(range(len(TILES)))

            def store_group(grp):
                ja, Pa, _, sa = tinfo[grp[0]]
                nsl = sum(tinfo[g][2] for g in grp)
                assert all(tinfo[g][1] == Pa for g in grp)
                nc.scalar.dma_start(
                    out_d[ja * BLK8: ja * BLK8 + nsl * Pa * BLK8].rearrange(
                        "(s p f) -> p s f", s=nsl, p=Pa, f=BLK8),
                    obfull[:Pa, sa * BLK8:(sa + nsl) * BLK8].rearrange(
                        "p (s f) -> p s f", s=nsl, f=BLK8))

            last_tile_to_group = {grp[-1]: grp for grp in STORE_GROUPS}

            T = len(tiles)
            LOOKAHEAD = 5  # loads run this many tiles ahead of back-stage
            for i in range(min(LOOKAHEAD, T)):
                tiles[i][0]()
            tiles[0][1]()          # front_0
            tiles[0][2]()          # mid_0
            for k in range(T):
                if k + LOOKAHEAD < T:
                    tiles[k + LOOKAHEAD][0]()   # load ahead
                tiles[k][3]()                   # back_k      (DVE)
                tiles[k][4]()                   # out_k       (Pool+Act)
                if k in last_tile_to_group:
                    store_group(last_tile_to_group[k])
                if k + 1 < T:
                    tiles[k + 1][1]()           # front_{k+1} (DVE)
                    tiles[k + 1][2]()           # mid_{k+1}   (DVE+Pool)

    nc.compile()
    return nc


def get_nc():
    if "nc" not in _CACHE:
        _CACHE["nc"] = _build_nc()
    return _CACHE["nc"]


def _check_maps(map_rows, map_cols):
    """The device program hardcodes the clip(4i-1..4i+2) scatter footprint;
    verify the provided maps match it exactly."""
    off = np.arange(4)
    rows = np.clip(4 * np.arange(Hp)[:, None] - 1 + off[None, :], 0, H - 1)
    cols = np.clip(4 * np.arange(Wp)[:, None] - 1 + off[None, :], 0, W - 1)
    exp_rows = np.broadcast_to(rows[:, None, :, None], (Hp, Wp, 4, 4)).reshape(Hp, Wp, 16)
    exp_cols = np.broadcast_to(cols[None, :, None, :], (Hp, Wp, 4, 4)).reshape(Hp, Wp, 16)
    if not (np.asarray(map_rows) == exp_rows).all() or \
       not (np.asarray(map_cols) == exp_cols).all():
        raise ValueError("map_rows/map_cols do not match the expected "
                         "clip(4i-1..4i+2) footprint this kernel hardcodes")


def kernel(x, lower_bound1, q1, map_rows, map_cols):
    from concourse.bass_utils import run_bass_kernel_spmd

    x = np.asarray(x, dtype=np.float32)
    lb = np.ascontiguousarray(np.asarray(lower_bound1, dtype=np.float32))
    q1 = np.ascontiguousarray(np.asarray(q1, dtype=np.float32))
    _check_maps(map_rows, map_cols)
    assert x.shape == (B, 1, H, W), x.shape

    thr4 = (np.float32(4.0) * (q1 / lb).astype(np.float32)).astype(np.float32)
    lbx, thrx = make_tables(lb, thr4)

    nc = get_nc()
    in_maps = [
        {"xp": make_stream(x[c * BC:(c + 1) * BC]), "lbx": lbx, "thrx": thrx}
        for c in range(NCORES)
    ]
    res = run_bass_kernel_spmd(nc, in_maps, list(range(NCORES)))
    out = np.concatenate([unstream_out(r["out"]) for r in res.results], axis=0)
    return np.ascontiguousarray(out.reshape(B, 1, H, W).astype(np.float32))
